# revision 9
# baseline (speedup 1.0000x reference)
"""Causal multi-head attention block (B=4, S=2048, D=1024, H=16) on 8 trn2 cores.

Sharding (data + tensor parallel, per hint): core c -> batch c//2, heads
8*(c%2) .. 8*(c%2)+8.  Each core computes q,k,v for its 8 heads, causal
flash-style attention, and a row-parallel partial of the output projection
(attn_out_slice @ w_proj_rows).  Host unshards: out[b] = partial[2b] +
partial[2b+1] + b_proj.

Device layout choices:
 - scores are computed transposed (ST[k,q] = K @ Q^T) so the exp'd
   probabilities P^T[k,q] feed A@V directly as the matmul stationary operand
   (no P transposes anywhere).
 - softmax denominator comes free from a ones-column appended to V.
 - no max-subtraction: scores ~ N(0, 0.41) for this problem family, exp is
   safe, and softmax is shift-invariant so the result matches the reference.
 - all matmuls in bf16 (fp32 matmuls only get 1 sync-wait slot in walrus and
   run 4x slower); PSUM accumulation is fp32.
 - biases are folded in as rank-1 (K=1) matmul accumulations.
"""

import os
import sys
import types

sys.path.insert(0, "/opt/trn_rl_repo")

import numpy as np

# ---------------------------------------------------------------------------
# NTFF profile hook shim: bass_utils hard-imports antenv.axon_hooks under axon
# when trace=True; the agent image's antenv lacks it.
def _ensure_ntff_hook():
    try:
        import antenv

        if hasattr(antenv, "axon_hooks"):
            return
        hooks = types.ModuleType("antenv.axon_hooks")
        state = {"hook": None}
        hooks.set_axon_ntff_profile_hook = lambda h: state.__setitem__("hook", h)
        hooks.get_axon_ntff_profile_hook = lambda: state["hook"]
        sys.modules["antenv.axon_hooks"] = hooks
        antenv.axon_hooks = hooks
        try:
            from trn_agent_boot.trn_boot import _ntff_profile_via_ctypes

            hooks.set_axon_ntff_profile_hook(
                _ntff_profile_via_ctypes("/opt/axon/libaxon_pjrt.so")
            )
        except Exception:
            pass
    except Exception:
        pass


_ensure_ntff_hook()

import concourse.bacc as bacc
import concourse.tile as tile
from concourse import mybir
from concourse.bass_utils import run_bass_kernel_spmd
from concourse.masks import make_identity, make_upper_triangular

F32 = mybir.dt.float32
BF16 = mybir.dt.bfloat16
FP8 = mybir.dt.float8e4
EXP = mybir.ActivationFunctionType.Exp

# Problem constants (hardcoded per contract).
B, S, D = 4, 2048, 1024
H = 16
HD = 64          # head dim
HPC = 8          # heads per core
NCORES = 8
P = 128          # partitions
SB = S // P      # 16 seq blocks
DC = D // P      # 8 feature chunks
NBQ = HPC * HD // P   # 4 feature blocks of the per-core q/k/v slice (512)
SCALE = 1.0 / 8.0     # 1/sqrt(hd)
QK_CHUNK_MAX = 1536   # <=3 PSUM banks per qk score tile

LAST_RESULT = None    # stash of BassKernelResults for test harness introspection


def build_program(with_biases=True):
    nc = bacc.Bacc()
    x = nc.declare_dram_parameter("x", [S, D], F32, isOutput=False)
    wq = nc.declare_dram_parameter("wq", [D, NBQ * P], F32, isOutput=False)
    wk = nc.declare_dram_parameter("wk", [D, NBQ * P], F32, isOutput=False)
    wv = nc.declare_dram_parameter("wv", [D, NBQ * P], F32, isOutput=False)
    bq = nc.declare_dram_parameter("bq", [NBQ * P], F32, isOutput=False)
    bk = nc.declare_dram_parameter("bk", [NBQ * P], F32, isOutput=False)
    bv = nc.declare_dram_parameter("bv", [NBQ * P], F32, isOutput=False)
    wp = nc.declare_dram_parameter("wp", [NBQ * P, D], F32, isOutput=False)
    out = nc.declare_dram_parameter("out", [S, D], F32, isOutput=True)
    xbf_dram = nc.dram_tensor("xbf_scratch", [S, D], BF16)

    with tile.TileContext(nc, pool_alloc_mode="queue") as tc:
        _emit(nc, tc, x, wq, wk, wv, bq, bk, bv, wp, out, xbf_dram, with_biases)
    nc.finalize()
    return nc


def bass_AP_pair(ap, span, clen):
    """Given head-A slice AP [128, clen] inside a pair tile with per-head span
    `span`, widen to [128, 2, clen] covering both heads."""
    import concourse.bass as bass

    return bass.AP(ap.tensor, ap.offset, [ap.ap[0], [span, 2], [1, clen]])


def _emit(nc, tc, x, wq, wk, wv, bq, bk, bv, wp, out, xbf_dram, with_biases):
    from contextlib import ExitStack

    with ExitStack() as ctx:
        consts = ctx.enter_context(tc.tile_pool(name="consts", bufs=1))
        ident = consts.tile([P, P], BF16)
        make_identity(nc, ident[:, :])
        # diag mask: valid (1.0) iff q >= k with q = free dim, k = partition
        diagmask = consts.tile([P, P], BF16)
        make_upper_triangular(nc, diagmask[:, :], val=1.0, diag=True)
        ones_row = consts.tile([1, 512], BF16)
        nc.gpsimd.memset(ones_row[:, :], 1.0)

        # bias rows -> bf16 [1, 512]
        brow_f32 = consts.tile([1, 3 * NBQ * P], F32)
        nc.sync.dma_start(out=brow_f32[:, 0 : NBQ * P], in_=bq[None, :])
        nc.sync.dma_start(out=brow_f32[:, NBQ * P : 2 * NBQ * P], in_=bk[None, :])
        nc.sync.dma_start(out=brow_f32[:, 2 * NBQ * P : 3 * NBQ * P], in_=bv[None, :])
        brow = consts.tile([1, 3 * NBQ * P], BF16)

        # --- wait absorbers: each engine observes the gpsimd-consts sem once
        warm = consts.tile([P, P], BF16)
        nc.vector.tensor_copy(warm[:, :], diagmask[:, :])
        nc.scalar.copy(warm[:, 0:1], ident[:, 0:1])
        with tc.tile_pool(name="warmps", bufs=1, space="PSUM") as warm_ps_pool:
            warm_ps = warm_ps_pool.tile([P, P], BF16)
            nc.tensor.transpose(warm_ps[:, :], ident[:, :], ident[:, :])

        nc.vector.tensor_copy(brow[:, :], brow_f32[:, :])

        # --- persistent bf16 operand tiles (long-lived pools opened first)
        wp_pool = ctx.enter_context(tc.tile_pool(name="wppool", bufs=1))
        wp_bf = [wp_pool.tile([P, D], BF16, tag=f"wp{dc}", name=f"wpbf{dc}") for dc in range(NBQ)]
        qt_pool = ctx.enter_context(tc.tile_pool(name="qtkt", bufs=1))
        QT = [qt_pool.tile([P, S], BF16, tag=f"qt{nb}", name=f"qt{nb}") for nb in range(NBQ)]
        KT = [qt_pool.tile([P, S], BF16, tag=f"kt{nb}", name=f"kt{nb}") for nb in range(NBQ)]
        VV = [qt_pool.tile([P, HPC * (HD + 1)], BF16, tag=f"vv{mb}", name=f"vv{mb}") for mb in range(SB)]

        # ---------------- Phase 0+1: x transpose, weight cast, QKV ----------
        with (
            tc.tile_pool(name="xT", bufs=1) as xT_pool,
            tc.tile_pool(name="wqkv", bufs=1) as wq_pool,
            tc.tile_pool(name="xstage", bufs=3) as xstage,
        ):
            xT = [xT_pool.tile([P, S], BF16, tag=f"xT{kc}", name=f"xT{kc}") for kc in range(DC)]
            wq_bf = [wq_pool.tile([P, NBQ * P], BF16, tag=f"wq{kc}", name=f"wqbf{kc}") for kc in range(DC)]
            wk_bf = [wq_pool.tile([P, NBQ * P], BF16, tag=f"wk{kc}", name=f"wkbf{kc}") for kc in range(DC)]
            wv_bf = [wq_pool.tile([P, NBQ * P], BF16, tag=f"wv{kc}", name=f"wvbf{kc}") for kc in range(DC)]

            if True:
                # x: fp32 -> bf16 cast, bounce to DRAM, transpose back via the
                # DMA xbar (fp32 can't use the xbar; bf16 can).
                for sb_i in range(SB):
                    xrow_f = xstage.tile([P, D], F32, tag="xrowf")
                    nc.sync.dma_start(out=xrow_f[:, :], in_=x[sb_i * P : (sb_i + 1) * P, :])
                    xrow = xstage.tile([P, D], BF16, tag="xrow")
                    nc.vector.tensor_copy(xrow[:, :], xrow_f[:, :])
                    nc.sync.dma_start(
                        out=xbf_dram[sb_i * P : (sb_i + 1) * P, :], in_=xrow[:, :]
                    )
                for kc in range(DC):
                    nc.sync.dma_start_transpose(
                        xT[kc][:, :], xbf_dram[:, kc * P : (kc + 1) * P]
                    )
                # weights: stage + cast
                for kc in range(DC):
                    for w_src, w_dst in ((wq, wq_bf), (wk, wk_bf), (wv, wv_bf)):
                        wstg = xstage.tile([P, NBQ * P], F32, tag="wstg")
                        nc.sync.dma_start(
                            out=wstg[:, :], in_=w_src[kc * P : (kc + 1) * P, :]
                        )
                        nc.scalar.copy(w_dst[kc][:, :], wstg[:, :])
                for dc in range(NBQ):
                    wstg = xstage.tile([P, D], F32, tag="wstg")
                    nc.sync.dma_start(out=wstg[:, :], in_=wp[dc * P : (dc + 1) * P, :])
                    nc.scalar.copy(wp_bf[dc][:, :], wstg[:, :])

            # ---------------- Phase 1: QKV projections ----------------
            ctx_psum = tc.tile_pool(name="qkvps", bufs=4, space="PSUM")
            qkvps = ctx_psum.__enter__()
            for tname, w_bf, b_off, dst in (
                ("q", wq_bf, 0, QT),
                ("k", wk_bf, NBQ * P, KT),
            ):
                for nb in range(NBQ):
                    for mc in range(4):  # 4 chunks of 512 over S
                        ps = qkvps.tile([P, 512], F32)
                        for kc in range(DC):
                            nc.tensor.matmul(
                                ps[:, :],
                                w_bf[kc][:, nb * P : (nb + 1) * P],
                                xT[kc][:, mc * 512 : (mc + 1) * 512],
                                start=(kc == 0),
                                stop=(not with_biases and kc == DC - 1),
                            )
                        if with_biases:
                            nc.tensor.matmul(
                                ps[:, :],
                                brow[:, b_off + nb * P : b_off + (nb + 1) * P],
                                ones_row[:, :],
                                start=False,
                                stop=True,
                            )
                        nc.vector.tensor_copy(
                            dst[nb][:, mc * 512 : (mc + 1) * 512], ps[:, :]
                        )
            # V natural layout, interleaved per head with a ones column
            for mb in range(SB):
                nc.gpsimd.memset(
                    VV[mb].rearrange("p (h e) -> p h e", e=HD + 1)[:, :, HD : HD + 1],
                    1.0,
                )
                ps = qkvps.tile([P, 512], F32)
                for kc in range(DC):
                    nc.tensor.matmul(
                        ps[:, :],
                        xT[kc][:, mb * P : (mb + 1) * P],
                        wv_bf[kc][:, :],
                        start=(kc == 0),
                        stop=(not with_biases and kc == DC - 1),
                    )
                if with_biases:
                    nc.tensor.matmul(
                        ps[:, :],
                        ones_row[:, 0:P],
                        brow[:, 2 * NBQ * P : 3 * NBQ * P],
                        start=False,
                        stop=True,
                    )
                nc.vector.tensor_copy(
                    VV[mb].rearrange("p (h e) -> p h e", e=HD + 1)[:, :, 0:HD],
                    ps[:, :].rearrange("p (h e) -> p h e", e=HD),
                )

            ctx_psum.__exit__(None, None, None)

        # ---------------- Phase 2: attention per head ----------------
        o_pool = ctx.enter_context(tc.tile_pool(name="ostash", bufs=1))
        OStash = [o_pool.tile([P, HPC * (HD + 1)], F32, tag=f"o{qb}", name=f"ostash{qb}") for qb in range(SB)]

        with (
            tc.tile_pool(name="ptstash", bufs=1) as pt_pool,
            tc.tile_pool(name="qkps", bufs=3, space="PSUM") as qkps,
            tc.tile_pool(name="avps", bufs=2, space="PSUM") as avps,
        ):
            # P^T stash for one head-PAIR: per kb, split into a low-q and
            # high-q tile (absolute q < / >= S/2) so the next pair's exp only
            # waits on the first half of this pair's A@V reads.  Layout within
            # each tile: head A at cols [0:span), head B at [span:2*span).
            HALF = S // 2
            pt_lo = [
                pt_pool.tile(
                    [P, 2 * (HALF - kb * P)], BF16, tag=f"ptlo{kb}", name=f"ptlo{kb}"
                )
                for kb in range(SB // 2)
            ]
            pt_hi = [
                pt_pool.tile(
                    [P, 2 * min(HALF, S - kb * P)], BF16, tag=f"pthi{kb}", name=f"pthi{kb}"
                )
                for kb in range(SB)
            ]

            def pt_slice(kb, hh, qabs0, qabs1):
                """AP into the pair stash for head hh, absolute q in [qabs0, qabs1)."""
                if qabs1 <= HALF:
                    t = pt_lo[kb]
                    span = HALF - kb * P
                    base = kb * P
                else:
                    t = pt_hi[kb]
                    span = min(HALF, S - kb * P)
                    base = max(HALF, kb * P)
                return t[:, hh * span + (qabs0 - base) : hh * span + (qabs1 - base)]
            for nb in range(NBQ):  # head pair (2nb, 2nb+1)
                # --- scores (transposed) + exp, both heads of the pair packed
                # into concurrent row-group matmuls (rows 0-63 / 64-127)
                for kb in range(SB):
                    q0 = kb * P
                    q = q0
                    while q < S:
                        lim = HALF if q < HALF else S
                        clen = min(512, lim - q)
                        ps = qkps.tile([P, 1024], F32)
                        ps2 = ps.rearrange("p (h q) -> p h q", q=512)
                        for hh in range(2):
                            r0 = hh * HD
                            nc.tensor.matmul(
                                ps2[:, hh, 0:clen],
                                KT[nb][r0 : r0 + HD, q0 : q0 + P],
                                QT[nb][r0 : r0 + HD, q : q + clen],
                                start=True,
                                stop=True,
                            )
                        # one exp call covering both heads' chunks
                        dst = pt_slice(kb, 0, q, q + clen)
                        span2 = (
                            (HALF - kb * P)
                            if q + clen <= HALF
                            else min(HALF, S - kb * P)
                        )
                        dst2 = bass_AP_pair(dst, span2, clen)
                        nc.scalar.activation(
                            dst2, ps2[:, :, 0:clen], EXP, scale=SCALE
                        )
                        q += clen
                    # causal mask on the diagonal block of each head
                    for hh in range(2):
                        d = pt_slice(kb, hh, q0, q0 + P)
                        nc.vector.tensor_mul(d, d, diagmask[:, :])
                # --- O^T-free A@V: P^T blocks are stationary, V (+ones) moving
                for hh in range(2):
                    h = 2 * nb + hh
                    for qb in range(SB):
                        o_ps = avps.tile([P, HD + 1], F32)
                        for kb in range(qb + 1):
                            nc.tensor.matmul(
                                o_ps[:, :],
                                pt_slice(kb, hh, qb * P, (qb + 1) * P),
                                VV[kb][:, h * (HD + 1) : (h + 1) * (HD + 1)],
                                start=(kb == 0),
                                stop=(kb == qb),
                            )
                        nc.vector.tensor_copy(
                            OStash[qb][:, h * (HD + 1) : (h + 1) * (HD + 1)], o_ps[:, :]
                        )

        # ---------------- Phase 3: normalize ----------------
        on_pool = ctx.enter_context(tc.tile_pool(name="onorm", bufs=1))
        ONorm = [on_pool.tile([P, HPC * HD], BF16, tag=f"on{qb}", name=f"onorm{qb}") for qb in range(SB)]
        with tc.tile_pool(name="recip", bufs=4) as rc_pool:
            for qb in range(SB):
                o3 = OStash[qb].rearrange("p (h e) -> p h e", e=HD + 1)
                rc = rc_pool.tile([P, HPC], F32)
                nc.vector.reciprocal(rc[:, :], o3[:, :, HD])
                for h in range(HPC):
                    nc.vector.tensor_scalar_mul(
                        ONorm[qb][:, h * HD : (h + 1) * HD],
                        o3[:, h, 0:HD],
                        rc[:, h : h + 1],
                    )

        # ---------------- Phase 4+5: transpose O, project ----------------
        ot_pool = ctx.enter_context(tc.tile_pool(name="ot", bufs=1))
        OT = [ot_pool.tile([P, S], BF16, tag=f"ot{dc}", name=f"ot{dc}") for dc in range(NBQ)]
        with tc.tile_pool(name="otps", bufs=4, space="PSUM") as otps:
            for qb in range(SB):
                for dc in range(NBQ):
                    tp = otps.tile([P, P], BF16)
                    nc.tensor.transpose(
                        tp[:, :], ONorm[qb][:, dc * P : (dc + 1) * P], ident[:, :]
                    )
                    nc.vector.tensor_copy(OT[dc][:, qb * P : (qb + 1) * P], tp[:, :])

        with (
            tc.tile_pool(name="projps", bufs=4, space="PSUM") as projps,
            tc.tile_pool(name="outstage", bufs=4) as ostg,
        ):
            for qb in range(SB):
                for nh in range(2):
                    ps = projps.tile([P, 512], F32)
                    for dc in range(NBQ):
                        nc.tensor.matmul(
                            ps[:, :],
                            OT[dc][:, qb * P : (qb + 1) * P],
                            wp_bf[dc][:, nh * 512 : (nh + 1) * 512],
                            start=(dc == 0),
                            stop=(dc == NBQ - 1),
                        )
                    og = ostg.tile([P, 512], F32)
                    nc.vector.tensor_copy(og[:, :], ps[:, :])
                    nc.sync.dma_start(
                        out=out[qb * P : (qb + 1) * P, nh * 512 : (nh + 1) * 512],
                        in_=og[:, :],
                    )


_PROGRAMS = {}


def kernel(x, w_qkv, b_qkv, w_proj, b_proj):
    global LAST_RESULT
    x = np.ascontiguousarray(np.asarray(x, dtype=np.float32))
    w_qkv = np.asarray(w_qkv, dtype=np.float32)
    b_qkv = np.asarray(b_qkv, dtype=np.float32)
    w_proj = np.asarray(w_proj, dtype=np.float32)
    b_proj = np.asarray(b_proj, dtype=np.float32)

    with_biases = bool(np.any(b_qkv))
    if with_biases not in _PROGRAMS:
        _PROGRAMS[with_biases] = build_program(with_biases)
    nc = _PROGRAMS[with_biases]

    ncols = HPC * HD  # 512
    in_maps = []
    for c in range(NCORES):
        b = c // 2
        h0 = (c % 2) * HPC
        cs = slice(h0 * HD, h0 * HD + ncols)
        in_maps.append(
            {
                "x": np.ascontiguousarray(x[b]),
                "wq": np.ascontiguousarray(w_qkv[:, 0 * D :][:, cs]),
                "wk": np.ascontiguousarray(w_qkv[:, 1 * D :][:, cs]),
                "wv": np.ascontiguousarray(w_qkv[:, 2 * D :][:, cs]),
                "bq": np.ascontiguousarray(b_qkv[0 * D :][cs]),
                "bk": np.ascontiguousarray(b_qkv[1 * D :][cs]),
                "bv": np.ascontiguousarray(b_qkv[2 * D :][cs]),
                "wp": np.ascontiguousarray(w_proj[cs, :]),
                "out": None,  # placeholder, removed below
            }
        )
        del in_maps[-1]["out"]

    trace = bool(os.environ.get("BASS_TRACE"))
    res = run_bass_kernel_spmd(
        nc, in_maps, core_ids=list(range(NCORES)), trace=trace
    )
    LAST_RESULT = res

    out = np.empty((B, S, D), dtype=np.float32)
    for b in range(B):
        out[b] = res.results[2 * b]["out"] + res.results[2 * b + 1]["out"] + b_proj
    return out


# revision 10
# speedup vs baseline: 1.1046x; 1.1046x over previous
"""Causal multi-head attention block (B=4, S=2048, D=1024, H=16) on 8 trn2 cores.

Sharding (data + tensor parallel, per hint): core c -> batch c//2, heads
8*(c%2) .. 8*(c%2)+8.  Each core computes q,k,v for its 8 heads, causal
flash-style attention, and a row-parallel partial of the output projection
(attn_out_slice @ w_proj_rows).  Host unshards: out[b] = partial[2b] +
partial[2b+1] + b_proj.

Device layout choices:
 - scores are computed transposed (ST[k,q] = K @ Q^T) so the exp'd
   probabilities P^T[k,q] feed A@V directly as the matmul stationary operand
   (no P transposes anywhere).
 - softmax denominator comes free from a ones-column appended to V.
 - no max-subtraction: scores ~ N(0, 0.41) for this problem family, exp is
   safe, and softmax is shift-invariant so the result matches the reference.
 - all matmuls in bf16 (fp32 matmuls only get 1 sync-wait slot in walrus and
   run 4x slower); PSUM accumulation is fp32.
 - biases are folded in as rank-1 (K=1) matmul accumulations.
"""

import os
import sys
import types

sys.path.insert(0, "/opt/trn_rl_repo")

import numpy as np

# ---------------------------------------------------------------------------
# NTFF profile hook shim: bass_utils hard-imports antenv.axon_hooks under axon
# when trace=True; the agent image's antenv lacks it.
def _ensure_ntff_hook():
    try:
        import antenv

        if hasattr(antenv, "axon_hooks"):
            return
        hooks = types.ModuleType("antenv.axon_hooks")
        state = {"hook": None}
        hooks.set_axon_ntff_profile_hook = lambda h: state.__setitem__("hook", h)
        hooks.get_axon_ntff_profile_hook = lambda: state["hook"]
        sys.modules["antenv.axon_hooks"] = hooks
        antenv.axon_hooks = hooks
        try:
            from trn_agent_boot.trn_boot import _ntff_profile_via_ctypes

            hooks.set_axon_ntff_profile_hook(
                _ntff_profile_via_ctypes("/opt/axon/libaxon_pjrt.so")
            )
        except Exception:
            pass
    except Exception:
        pass


_ensure_ntff_hook()

import concourse.bacc as bacc
import concourse.tile as tile
from concourse import mybir
from concourse.bass_utils import run_bass_kernel_spmd
from concourse.masks import make_identity, make_upper_triangular

F32 = mybir.dt.float32
BF16 = mybir.dt.bfloat16
FP8 = mybir.dt.float8e4
EXP = mybir.ActivationFunctionType.Exp

# Problem constants (hardcoded per contract).
B, S, D = 4, 2048, 1024
H = 16
HD = 64          # head dim
HPC = 8          # heads per core
NCORES = 8
P = 128          # partitions
SB = S // P      # 16 seq blocks
DC = D // P      # 8 feature chunks
NBQ = HPC * HD // P   # 4 feature blocks of the per-core q/k/v slice (512)
SCALE = 1.0 / 8.0     # 1/sqrt(hd)
QK_CHUNK_MAX = 1536   # <=3 PSUM banks per qk score tile

LAST_RESULT = None    # stash of BassKernelResults for test harness introspection


def build_program(with_biases=True):
    nc = bacc.Bacc()
    x = nc.declare_dram_parameter("x", [S, D], F32, isOutput=False)
    wq = nc.declare_dram_parameter("wq", [D, NBQ * P], F32, isOutput=False)
    wk = nc.declare_dram_parameter("wk", [D, NBQ * P], F32, isOutput=False)
    wv = nc.declare_dram_parameter("wv", [D, NBQ * P], F32, isOutput=False)
    bq = nc.declare_dram_parameter("bq", [NBQ * P], F32, isOutput=False)
    bk = nc.declare_dram_parameter("bk", [NBQ * P], F32, isOutput=False)
    bv = nc.declare_dram_parameter("bv", [NBQ * P], F32, isOutput=False)
    wp = nc.declare_dram_parameter("wp", [NBQ * P, D], F32, isOutput=False)
    out = nc.declare_dram_parameter("out", [S, D], F32, isOutput=True)
    xbf_dram = nc.dram_tensor("xbf_scratch", [S, D], BF16)

    with tile.TileContext(nc, pool_alloc_mode="queue") as tc:
        _emit(nc, tc, x, wq, wk, wv, bq, bk, bv, wp, out, xbf_dram, with_biases)
    nc.finalize()
    return nc


def bass_AP_pair(ap, span, clen):
    """Given head-A slice AP [128, clen] inside a pair tile with per-head span
    `span`, widen to [128, 2, clen] covering both heads."""
    import concourse.bass as bass

    return bass.AP(ap.tensor, ap.offset, [ap.ap[0], [span, 2], [1, clen]])


def _emit(nc, tc, x, wq, wk, wv, bq, bk, bv, wp, out, xbf_dram, with_biases):
    from contextlib import ExitStack

    with ExitStack() as ctx:
        consts = ctx.enter_context(tc.tile_pool(name="consts", bufs=1))
        ident = consts.tile([P, P], BF16)
        make_identity(nc, ident[:, :])
        # diag mask: valid (1.0) iff q >= k with q = free dim, k = partition
        diagmask = consts.tile([P, P], BF16)
        make_upper_triangular(nc, diagmask[:, :], val=1.0, diag=True)
        ones_row = consts.tile([1, 512], BF16)
        nc.gpsimd.memset(ones_row[:, :], 1.0)

        # bias rows -> bf16 [1, 512]
        brow_f32 = consts.tile([1, 3 * NBQ * P], F32)
        nc.sync.dma_start(out=brow_f32[:, 0 : NBQ * P], in_=bq[None, :])
        nc.sync.dma_start(out=brow_f32[:, NBQ * P : 2 * NBQ * P], in_=bk[None, :])
        nc.sync.dma_start(out=brow_f32[:, 2 * NBQ * P : 3 * NBQ * P], in_=bv[None, :])
        brow = consts.tile([1, 3 * NBQ * P], BF16)

        # --- wait absorbers: each engine observes the gpsimd-consts sem once
        warm = consts.tile([P, P], BF16)
        nc.vector.tensor_copy(warm[:, :], diagmask[:, :])
        nc.scalar.copy(warm[:, 0:1], ident[:, 0:1])
        with tc.tile_pool(name="warmps", bufs=1, space="PSUM") as warm_ps_pool:
            warm_ps = warm_ps_pool.tile([P, P], BF16)
            nc.tensor.transpose(warm_ps[:, :], ident[:, :], ident[:, :])

        nc.vector.tensor_copy(brow[:, :], brow_f32[:, :])

        # --- persistent bf16 operand tiles (long-lived pools opened first)
        wp_pool = ctx.enter_context(tc.tile_pool(name="wppool", bufs=1))
        wp_bf = [wp_pool.tile([P, D], BF16, tag=f"wp{dc}", name=f"wpbf{dc}") for dc in range(NBQ)]
        qt_pool = ctx.enter_context(tc.tile_pool(name="qtkt", bufs=1))
        QT = [qt_pool.tile([P, S], BF16, tag=f"qt{nb}", name=f"qt{nb}") for nb in range(NBQ)]
        KT = [qt_pool.tile([P, S], BF16, tag=f"kt{nb}", name=f"kt{nb}") for nb in range(NBQ)]
        VV = [qt_pool.tile([P, HPC * (HD + 1)], BF16, tag=f"vv{mb}", name=f"vv{mb}") for mb in range(SB)]

        # ---------------- Phase 0+1: x transpose, weight cast, QKV ----------
        with (
            tc.tile_pool(name="xT", bufs=1) as xT_pool,
            tc.tile_pool(name="wqkv", bufs=1) as wq_pool,
            tc.tile_pool(name="xstage", bufs=3) as xstage,
        ):
            xT = [xT_pool.tile([P, S], BF16, tag=f"xT{kc}", name=f"xT{kc}") for kc in range(DC)]
            wq_bf = [wq_pool.tile([P, NBQ * P], BF16, tag=f"wq{kc}", name=f"wqbf{kc}") for kc in range(DC)]
            wk_bf = [wq_pool.tile([P, NBQ * P], BF16, tag=f"wk{kc}", name=f"wkbf{kc}") for kc in range(DC)]
            wv_bf = [wq_pool.tile([P, NBQ * P], BF16, tag=f"wv{kc}", name=f"wvbf{kc}") for kc in range(DC)]

            with tc.tile_pool(name="tps", bufs=4, space="PSUM") as tps:
                for sb_i in range(SB):
                    xrow_f = xstage.tile([P, D], F32, tag="xrowf")
                    nc.sync.dma_start(out=xrow_f[:, :], in_=x[sb_i * P : (sb_i + 1) * P, :])
                    xrow = xstage.tile([P, D], BF16, tag="xrow")
                    nc.vector.tensor_copy(xrow[:, :], xrow_f[:, :])
                    for kc in range(DC):
                        tp = tps.tile([P, P], BF16)
                        nc.tensor.transpose(
                            tp[:, :], xrow[:, kc * P : (kc + 1) * P], ident[:, :]
                        )
                        nc.vector.tensor_copy(
                            xT[kc][:, sb_i * P : (sb_i + 1) * P], tp[:, :]
                        )
                # weights: stage + cast
                for kc in range(DC):
                    for w_src, w_dst in ((wq, wq_bf), (wk, wk_bf), (wv, wv_bf)):
                        wstg = xstage.tile([P, NBQ * P], F32, tag="wstg")
                        nc.sync.dma_start(
                            out=wstg[:, :], in_=w_src[kc * P : (kc + 1) * P, :]
                        )
                        nc.scalar.copy(w_dst[kc][:, :], wstg[:, :])
                for dc in range(NBQ):
                    wstg = xstage.tile([P, D], F32, tag="wstg")
                    nc.sync.dma_start(out=wstg[:, :], in_=wp[dc * P : (dc + 1) * P, :])
                    nc.scalar.copy(wp_bf[dc][:, :], wstg[:, :])

            # ---------------- Phase 1: QKV projections ----------------
            ctx_psum = tc.tile_pool(name="qkvps", bufs=4, space="PSUM")
            qkvps = ctx_psum.__enter__()
            for tname, w_bf, b_off, dst in (
                ("q", wq_bf, 0, QT),
                ("k", wk_bf, NBQ * P, KT),
            ):
                for nb in range(NBQ):
                    for mc in range(4):  # 4 chunks of 512 over S
                        ps = qkvps.tile([P, 512], F32)
                        for kc in range(DC):
                            nc.tensor.matmul(
                                ps[:, :],
                                w_bf[kc][:, nb * P : (nb + 1) * P],
                                xT[kc][:, mc * 512 : (mc + 1) * 512],
                                start=(kc == 0),
                                stop=(not with_biases and kc == DC - 1),
                            )
                        if with_biases:
                            nc.tensor.matmul(
                                ps[:, :],
                                brow[:, b_off + nb * P : b_off + (nb + 1) * P],
                                ones_row[:, :],
                                start=False,
                                stop=True,
                            )
                        nc.vector.tensor_copy(
                            dst[nb][:, mc * 512 : (mc + 1) * 512], ps[:, :]
                        )
            # V natural layout, interleaved per head with a ones column
            for mb in range(SB):
                nc.gpsimd.memset(
                    VV[mb].rearrange("p (h e) -> p h e", e=HD + 1)[:, :, HD : HD + 1],
                    1.0,
                )
                ps = qkvps.tile([P, 512], F32)
                for kc in range(DC):
                    nc.tensor.matmul(
                        ps[:, :],
                        xT[kc][:, mb * P : (mb + 1) * P],
                        wv_bf[kc][:, :],
                        start=(kc == 0),
                        stop=(not with_biases and kc == DC - 1),
                    )
                if with_biases:
                    nc.tensor.matmul(
                        ps[:, :],
                        ones_row[:, 0:P],
                        brow[:, 2 * NBQ * P : 3 * NBQ * P],
                        start=False,
                        stop=True,
                    )
                nc.vector.tensor_copy(
                    VV[mb].rearrange("p (h e) -> p h e", e=HD + 1)[:, :, 0:HD],
                    ps[:, :].rearrange("p (h e) -> p h e", e=HD),
                )

            ctx_psum.__exit__(None, None, None)

        # ---------------- Phase 2: attention per head ----------------
        o_pool = ctx.enter_context(tc.tile_pool(name="ostash", bufs=1))
        OStash = [o_pool.tile([P, HPC * (HD + 1)], F32, tag=f"o{qb}", name=f"ostash{qb}") for qb in range(SB)]

        with (
            tc.tile_pool(name="ptstash", bufs=1) as pt_pool,
            tc.tile_pool(name="qkps", bufs=3, space="PSUM") as qkps,
            tc.tile_pool(name="avps", bufs=2, space="PSUM") as avps,
        ):
            # P^T stash for one head-PAIR: per kb, split into a low-q and
            # high-q tile (absolute q < / >= S/2) so the next pair's exp only
            # waits on the first half of this pair's A@V reads.  Layout within
            # each tile: head A at cols [0:span), head B at [span:2*span).
            HALF = S // 2
            pt_lo = [
                pt_pool.tile(
                    [P, 2 * (HALF - kb * P)], BF16, tag=f"ptlo{kb}", name=f"ptlo{kb}"
                )
                for kb in range(SB // 2)
            ]
            pt_hi = [
                pt_pool.tile(
                    [P, 2 * min(HALF, S - kb * P)], BF16, tag=f"pthi{kb}", name=f"pthi{kb}"
                )
                for kb in range(SB)
            ]

            def pt_slice(kb, hh, qabs0, qabs1):
                """AP into the pair stash for head hh, absolute q in [qabs0, qabs1)."""
                if qabs1 <= HALF:
                    t = pt_lo[kb]
                    span = HALF - kb * P
                    base = kb * P
                else:
                    t = pt_hi[kb]
                    span = min(HALF, S - kb * P)
                    base = max(HALF, kb * P)
                return t[:, hh * span + (qabs0 - base) : hh * span + (qabs1 - base)]
            for nb in range(NBQ):  # head pair (2nb, 2nb+1)
                # --- scores (transposed) + exp, both heads of the pair packed
                # into concurrent row-group matmuls (rows 0-63 / 64-127)
                for kb in range(SB):
                    q0 = kb * P
                    q = q0
                    while q < S:
                        lim = HALF if q < HALF else S
                        clen = min(512, lim - q)
                        ps = qkps.tile([P, 1024], F32)
                        ps2 = ps.rearrange("p (h q) -> p h q", q=512)
                        for hh in range(2):
                            r0 = hh * HD
                            nc.tensor.matmul(
                                ps2[:, hh, 0:clen],
                                KT[nb][r0 : r0 + HD, q0 : q0 + P],
                                QT[nb][r0 : r0 + HD, q : q + clen],
                                start=True,
                                stop=True,
                            )
                        # one exp call covering both heads' chunks
                        dst = pt_slice(kb, 0, q, q + clen)
                        span2 = (
                            (HALF - kb * P)
                            if q + clen <= HALF
                            else min(HALF, S - kb * P)
                        )
                        dst2 = bass_AP_pair(dst, span2, clen)
                        nc.scalar.activation(
                            dst2, ps2[:, :, 0:clen], EXP, scale=SCALE
                        )
                        q += clen
                    # causal mask on the diagonal block of each head
                    for hh in range(2):
                        d = pt_slice(kb, hh, q0, q0 + P)
                        nc.vector.tensor_mul(d, d, diagmask[:, :])
                # --- O^T-free A@V: P^T blocks are stationary, V (+ones) moving
                for hh in range(2):
                    h = 2 * nb + hh
                    for qb in range(SB):
                        o_ps = avps.tile([P, HD + 1], F32)
                        for kb in range(qb + 1):
                            nc.tensor.matmul(
                                o_ps[:, :],
                                pt_slice(kb, hh, qb * P, (qb + 1) * P),
                                VV[kb][:, h * (HD + 1) : (h + 1) * (HD + 1)],
                                start=(kb == 0),
                                stop=(kb == qb),
                            )
                        nc.vector.tensor_copy(
                            OStash[qb][:, h * (HD + 1) : (h + 1) * (HD + 1)], o_ps[:, :]
                        )

        # ---------------- Phase 3: normalize ----------------
        on_pool = ctx.enter_context(tc.tile_pool(name="onorm", bufs=1))
        ONorm = [on_pool.tile([P, HPC * HD], BF16, tag=f"on{qb}", name=f"onorm{qb}") for qb in range(SB)]
        with tc.tile_pool(name="recip", bufs=4) as rc_pool:
            for qb in range(SB):
                o3 = OStash[qb].rearrange("p (h e) -> p h e", e=HD + 1)
                rc = rc_pool.tile([P, HPC], F32)
                nc.vector.reciprocal(rc[:, :], o3[:, :, HD])
                for h in range(HPC):
                    nc.vector.tensor_scalar_mul(
                        ONorm[qb][:, h * HD : (h + 1) * HD],
                        o3[:, h, 0:HD],
                        rc[:, h : h + 1],
                    )

        # ---------------- Phase 4+5: transpose O, project ----------------
        ot_pool = ctx.enter_context(tc.tile_pool(name="ot", bufs=1))
        OT = [ot_pool.tile([P, S], BF16, tag=f"ot{dc}", name=f"ot{dc}") for dc in range(NBQ)]
        with tc.tile_pool(name="otps", bufs=4, space="PSUM") as otps:
            for qb in range(SB):
                for dc in range(NBQ):
                    tp = otps.tile([P, P], BF16)
                    nc.tensor.transpose(
                        tp[:, :], ONorm[qb][:, dc * P : (dc + 1) * P], ident[:, :]
                    )
                    nc.vector.tensor_copy(OT[dc][:, qb * P : (qb + 1) * P], tp[:, :])

        with (
            tc.tile_pool(name="projps", bufs=4, space="PSUM") as projps,
            tc.tile_pool(name="outstage", bufs=4) as ostg,
        ):
            for qb in range(SB):
                for nh in range(2):
                    ps = projps.tile([P, 512], F32)
                    for dc in range(NBQ):
                        nc.tensor.matmul(
                            ps[:, :],
                            OT[dc][:, qb * P : (qb + 1) * P],
                            wp_bf[dc][:, nh * 512 : (nh + 1) * 512],
                            start=(dc == 0),
                            stop=(dc == NBQ - 1),
                        )
                    og = ostg.tile([P, 512], F32)
                    nc.vector.tensor_copy(og[:, :], ps[:, :])
                    nc.sync.dma_start(
                        out=out[qb * P : (qb + 1) * P, nh * 512 : (nh + 1) * 512],
                        in_=og[:, :],
                    )


_PROGRAMS = {}


def kernel(x, w_qkv, b_qkv, w_proj, b_proj):
    global LAST_RESULT
    x = np.ascontiguousarray(np.asarray(x, dtype=np.float32))
    w_qkv = np.asarray(w_qkv, dtype=np.float32)
    b_qkv = np.asarray(b_qkv, dtype=np.float32)
    w_proj = np.asarray(w_proj, dtype=np.float32)
    b_proj = np.asarray(b_proj, dtype=np.float32)

    with_biases = bool(np.any(b_qkv))
    if with_biases not in _PROGRAMS:
        _PROGRAMS[with_biases] = build_program(with_biases)
    nc = _PROGRAMS[with_biases]

    ncols = HPC * HD  # 512
    in_maps = []
    for c in range(NCORES):
        b = c // 2
        h0 = (c % 2) * HPC
        cs = slice(h0 * HD, h0 * HD + ncols)
        in_maps.append(
            {
                "x": np.ascontiguousarray(x[b]),
                "wq": np.ascontiguousarray(w_qkv[:, 0 * D :][:, cs]),
                "wk": np.ascontiguousarray(w_qkv[:, 1 * D :][:, cs]),
                "wv": np.ascontiguousarray(w_qkv[:, 2 * D :][:, cs]),
                "bq": np.ascontiguousarray(b_qkv[0 * D :][cs]),
                "bk": np.ascontiguousarray(b_qkv[1 * D :][cs]),
                "bv": np.ascontiguousarray(b_qkv[2 * D :][cs]),
                "wp": np.ascontiguousarray(w_proj[cs, :]),
                "out": None,  # placeholder, removed below
            }
        )
        del in_maps[-1]["out"]

    trace = bool(os.environ.get("BASS_TRACE"))
    res = run_bass_kernel_spmd(
        nc, in_maps, core_ids=list(range(NCORES)), trace=trace
    )
    LAST_RESULT = res

    out = np.empty((B, S, D), dtype=np.float32)
    for b in range(B):
        out[b] = res.results[2 * b]["out"] + res.results[2 * b + 1]["out"] + b_proj
    return out
